# revision 3
# baseline (speedup 1.0000x reference)
"""TRN2 Bass kernel for nn_BeyazKusAIAttention_36515811951168.

Key reduction: the reference applies softmax over a size-1 axis, which is
identically 1.0, so attention weights are exactly 1 and the module collapses
to
    y = (x @ Wv^T) @ Wfold^T,  with  Wfold = Wo.reshape(4096,4,1024).sum(1)
(q/rope/scores/mask are dead code; `out` is v tiled over the 4 heads, and the
o-projection of the tiled v folds head-wise into Wfold).  This is a 5x FLOP
reduction vs the reference graph.

Execution: data-parallel over the 16384 = batch*seq rows across 8 NeuronCores
(no collectives).  All matmuls run in float32r (TF32-class multiply precision
with fp32 PSUM accumulation, full PE throughput at free-dim >= 256); measured
end-to-end relative error vs fp64 is ~2e-4.

Per-core program (R = 2048 rows):
  phase 1:  v^T = Wv @ x^T   - Wv^T resident in SBUF (16 MB), x^T streamed in
            row-chunks of 256, PSUM accumulation over K=4096 in 32 k-tiles,
            v^T staged to a DRAM scratch.
  phase 2:  y = v @ Wfold^T  - Wfold^T resident in SBUF (16 MB), v^T tiles
            are the stationary operand, K=1024 in 8 k-tiles, free-dim 512.

Host-side layouts (partition dim = contraction dim for both matmuls):
  xt [32,128,R]: xt[k,p,r] = x[row r, dim 128k+p]     (transposed shard)
  wvt[32,128,1024]: wvt[k,p,m] = Wv[m, 128k+p]
  wft[8,128,4096]:  wft[k,p,n] = Wfold[n, 128k+p]
  y  [R/128,128,4096]: y[t,p,n] = out[row 128t+p, n]
"""
import numpy as np
import concourse.bass as bass
from concourse import bacc
import concourse.mybir as mybir
from concourse.tile import TileContext
from concourse.bass_utils import run_bass_kernel_spmd

DIM = 4096
KV = 1024
N_CORES = 8
ROWS_TOTAL = 4 * 4096
ROWS = ROWS_TOTAL // N_CORES   # 2048
KT1 = DIM // 128               # 32 k-tiles, phase 1
MT1 = KV // 128                # 8 vcol tiles
KT2 = KV // 128                # 8 k-tiles, phase 2
NC2 = DIM // 512               # 8 ycol chunks
CHUNK1 = 256                   # phase-1 row-chunk width

_nc_cache = {}


def _build(rows=ROWS, chunk1=CHUNK1):
    nch = rows // chunk1
    mt2 = rows // 128
    f32, f32r = mybir.dt.float32, mybir.dt.float32r

    nc = bacc.Bacc(None, target_bir_lowering=False)
    XT = nc.dram_tensor("xt", [KT1, 128, rows], f32r, kind="ExternalInput")
    WVT = nc.dram_tensor("wvt", [KT1, 128, KV], f32r, kind="ExternalInput")
    WFT = nc.dram_tensor("wft", [KT2, 128, DIM], f32r, kind="ExternalInput")
    Y = nc.dram_tensor("y", [mt2, 128, DIM], f32, kind="ExternalOutput")

    with TileContext(nc) as tc:
        with tc.tile_pool(name="dram", bufs=1, space="DRAM") as dram:
            VT = dram.tile([KT2, 128, rows], f32r)

            # ---------------- phase 1: v^T = Wv @ x^T ----------------
            with (
                tc.tile_pool(name="wv", bufs=1) as wvpool,
                tc.tile_pool(name="xch", bufs=2) as xpool,
                tc.tile_pool(name="vst", bufs=4) as vpool,
                tc.tile_pool(name="ps1", bufs=2, space="PSUM") as pspool,
            ):
                wv = []
                for k in range(KT1):
                    wvk = wvpool.tile([128, KV], f32r, tag=f"wv{k}")
                    nc.sync.dma_start(wvk[:], WVT[k])
                    wv.append(wvk)
                for rc in range(nch):
                    xt = xpool.tile([128, KT1, chunk1], f32r, tag="xt")
                    for k in range(KT1):
                        nc.sync.dma_start(
                            xt[:, k, :],
                            XT[k, :, rc * chunk1:(rc + 1) * chunk1])
                    for m in range(MT1):
                        ps = pspool.tile([128, chunk1], f32, tag="ps")
                        for k in range(KT1):
                            nc.tensor.matmul(
                                ps[:], wv[k][:, m * 128:(m + 1) * 128],
                                xt[:, k, :],
                                start=(k == 0), stop=(k == KT1 - 1))
                        vs = vpool.tile([128, chunk1], f32r, tag="vs")
                        nc.vector.tensor_copy(vs[:], ps[:])
                        nc.sync.dma_start(
                            VT[m, :, rc * chunk1:(rc + 1) * chunk1], vs[:])

            # ---------------- phase 2: y = v @ Wfold^T ----------------
            with (
                tc.tile_pool(name="wf", bufs=1) as wfpool,
                tc.tile_pool(name="vtt", bufs=3) as vtpool,
                tc.tile_pool(name="yst", bufs=4) as ypool,
                tc.tile_pool(name="ps2", bufs=2, space="PSUM") as ps2pool,
            ):
                wf = []
                for n in range(NC2):
                    wfn = wfpool.tile([128, KT2, 512], f32r, tag=f"wf{n}")
                    for k in range(KT2):
                        nc.sync.dma_start(wfn[:, k, :],
                                          WFT[k, :, n * 512:(n + 1) * 512])
                    wf.append(wfn)
                for mt in range(mt2):
                    vt = vtpool.tile([128, KT2, 128], f32r, tag="vt")
                    for k in range(KT2):
                        nc.sync.dma_start(
                            vt[:, k, :], VT[k, :, mt * 128:(mt + 1) * 128])
                    for n in range(NC2):
                        ps = ps2pool.tile([128, 512], f32, tag="ps2")
                        for k in range(KT2):
                            nc.tensor.matmul(
                                ps[:], vt[:, k, :],
                                wf[n][:, k, :],
                                start=(k == 0), stop=(k == KT2 - 1))
                        ys = ypool.tile([128, 512], f32, tag="ys")
                        nc.vector.tensor_copy(ys[:], ps[:])
                        nc.sync.dma_start(Y[mt, :, n * 512:(n + 1) * 512],
                                          ys[:])
    nc.compile()
    return nc


def kernel(x, Wq, Wk, Wv, Wo, mask):
    x = np.asarray(x)
    Wv = np.asarray(Wv, dtype=np.float32)
    Wo = np.asarray(Wo, dtype=np.float32)
    B, S, D = x.shape
    assert D == DIM and B * S == ROWS_TOTAL

    # host-side relayout: transpose x once, fold Wo over heads
    x2 = np.ascontiguousarray(
        x.reshape(ROWS_TOTAL, DIM).T.astype(np.float32, copy=False))
    xt_all = x2.reshape(KT1, 128, ROWS_TOTAL)
    wvt = np.ascontiguousarray(Wv.T).reshape(KT1, 128, KV)
    wfold = Wo.reshape(DIM, 4, KV).sum(axis=1)
    wft = np.ascontiguousarray(wfold.T).reshape(KT2, 128, DIM)

    in_maps = []
    for c in range(N_CORES):
        in_maps.append({
            "xt": np.ascontiguousarray(
                xt_all[:, :, c * ROWS:(c + 1) * ROWS]),
            "wvt": wvt,
            "wft": wft,
        })

    if "nc" not in _nc_cache:
        _nc_cache["nc"] = _build()
    nc = _nc_cache["nc"]

    results = run_bass_kernel_spmd(nc, in_maps,
                                   core_ids=list(range(N_CORES))).results
    shards = [r["y"].reshape(ROWS, DIM) for r in results]
    out = np.concatenate(shards, axis=0).reshape(B, S, DIM)
    return out.astype(np.float32, copy=False)
